# revision 45
# baseline (speedup 1.0000x reference)
"""Trainium2 Bass kernel for nn_CGNN (3-layer GINE-style message-passing GNN).

Self-contained: takes FULL inputs (as produced by the problem's setup_inputs),
distributes across 8 NeuronCores internally (SPMD, one program, per-core data),
returns the FULL [64, 5] output.

Per-core design (v3 — batched SWDGE gathers + balanced node permutation):
  - nodes split into 8 equal ranges of N/8 (padded to a multiple of 128);
    each edge lives on the core owning its dst node.
  - per-core node permutation (_balance_perm): LPT-packs nodes into 128-node
    dst windows balancing indegree, then picks pair parities by greedy
    discrepancy so each (window, src-parity) class needs ~8 edge tiles
    (NT 880 -> 848); applied to table rows, dst windows, xT and h3.
  - edges grouped by 128-node dst windows; within each window edges are split
    by src parity (even/odd) and padded to 128-edge tiles. A layer's edge
    phase runs as two sweeps: all even tiles (window-major), then all odd
    tiles. Each sweep accumulates into its own SBUF aggT buffer, so PSUM
    bank-group lifetimes stay local to the sweep.
  - h table in DRAM is bf16 [n_nodes_pad, 64], viewed as pair rows
    [n/2, 128] (256B stride). Gathers use nc.gpsimd.dma_gather with int16
    indices = src>>1 (fits int16): even-src tiles gather from the base view,
    odd-src tiles from the +64-element offset view. GBT=8 tiles per call
    (994ns fixed SWDGE overhead amortized; 1024 idx = SWDGE ring max;
    measured ~8.6us/call = the Q7 descriptor-generation floor, gathp bufs=4
    keeps calls streaming).
  - stationary edge features staged k-major [K, NT, 128] so each block load
    is one 2KB descriptor per partition row (keeps SDMA off the gather path).
  - per edge tile: c = [ea; 1] @ [W'l; b'l] via per-tile PE matmul into a
    [128, 512] PSUM block (8 tiles); gathered h added via DVE; ACT relu ->
    m bf16.
  - scatter: aggT[:, win] += m^T via PE matmul with DVE-built one-hot.
  - node phase, feat-major: z-adds chunked into the MLP loop; MLP with BN
    folded into W2/b2; bias+relu fused in ACT. (f32r matmuls break the
    axon-path neuronxcc compile — keep fp32.)
  - next-layer table: bf16 cast + DMA-transpose + AllGather per chunk
    (0,2048,4096,5760,6272) — small last chunk so the final AllGather,
    which gates the next layer's gathers, lands early. The per-layer
    AllGather chain is serialized on the CC path (~81us).
  - pooling + the tiny 2-layer head run on host from returned h3 slices.
"""
import os
import sys
import numpy as np

sys.path.insert(0, "/opt/trn_rl_repo")

import ml_dtypes  # noqa: E402


def _install_ntff_shim(so_path="/opt/axon/libaxon_pjrt.so"):
    """Register the axon NTFF profile hook so trace=True works (optional)."""
    import types, contextlib, ctypes
    try:
        lib = ctypes.CDLL(so_path)
        if not hasattr(lib, "axon_start_nrt_profile"):
            return False
        lib.axon_start_nrt_profile.argtypes = [ctypes.POINTER(ctypes.c_int64),
                                               ctypes.c_size_t]
        lib.axon_start_nrt_profile.restype = ctypes.c_int64
        lib.axon_stop_nrt_profile.argtypes = [ctypes.c_char_p]
        lib.axon_stop_nrt_profile.restype = ctypes.c_int64

        @contextlib.contextmanager
        def _hook(output_dir, device_ids):
            import jax
            jax.devices()
            if device_ids:
                ids = (ctypes.c_int64 * len(device_ids))(*device_ids)
                rc = lib.axon_start_nrt_profile(ids, len(device_ids))
            else:
                rc = lib.axon_start_nrt_profile(None, 0)
            if rc != 0:
                raise RuntimeError(f"axon_start_nrt_profile rc={rc}")
            try:
                yield
            finally:
                n = lib.axon_stop_nrt_profile(str(output_dir).encode())
                if n < 0:
                    raise RuntimeError(f"axon_stop_nrt_profile rc={n}")

        import antenv
        mod = types.ModuleType("antenv.axon_hooks")
        mod.get_axon_ntff_profile_hook = lambda: _hook
        mod.set_axon_ntff_profile_hook = lambda h: None
        sys.modules["antenv.axon_hooks"] = mod
        antenv.axon_hooks = mod
        return True
    except Exception:
        return False

N_NODES = 50000
HID = 64
N_LAYERS = 3
N_CLASSES = 5
N_GRAPHS = 64
BN_EPS = 1e-5
N_CORES = 8
GBT = int(__import__('os').environ.get('GBT', 8))  # tiles per batched dma_gather call

BF16 = ml_dtypes.bfloat16


class Cfg:
    def __init__(self, n_nodes=N_NODES, n_cores=N_CORES):
        assert n_nodes % n_cores == 0
        self.n_nodes = n_nodes
        self.n_cores = n_cores
        self.own = n_nodes // n_cores
        self.own_pad = ((self.own + 127) // 128) * 128
        self.n_win = self.own_pad // 128
        self.tab_rows = n_cores * self.own_pad  # even; +2 pad rows in tensor
        # chunk-major table layout boundaries == AllGather chunks; last chunk
        # small so the final AllGather (which gates the next layer's gathers)
        # completes right after the node phase
        assert self.own_pad == 6272
        self.chunks = (0, 2048, 4096, 5760, self.own_pad)


def _chunks(total, step):
    out, a = [], 0
    while a < total:
        out.append((a, min(a + step, total)))
        a += step
    return out


def _wrap16(idx_flat):
    """SWDGE index layout: index g at [g%16, g//16], replicated over the
    8 gpsimd core groups (partitions 16a+r)."""
    n = idx_flat.shape[0]
    a = np.zeros((16, n // 16), np.int16)
    a[np.arange(n) % 16, np.arange(n) // 16] = idx_flat
    return np.tile(a, (8, 1))


# =============================================================== host prep
def _balance_perm(cfg, src, dst):
    """Per-core node permutation: LPT-pack nodes into 128-node windows so
    per-window indegree is balanced, then choose pair parities (greedy
    discrepancy) so each window's even/odd source split is balanced. Nearly
    every (window, parity) class then needs exactly 8 edge tiles.
    Returns pos: node -> local position (0..own_pad-1) within its core."""
    import heapq
    own, own_pad, nw = cfg.own, cfg.own_pad, cfg.n_win
    n = cfg.n_nodes
    indeg = np.bincount(dst, minlength=n)
    pos = np.zeros(n, np.int64)
    members = np.full((cfg.n_cores, nw, 128), -1, np.int64)
    fill = np.zeros((cfg.n_cores, nw), np.int64)
    # last window = spill: the 128 highest-indegree nodes (aligned across
    # cores so the per-class max stays tight); LPT the rest into 0..nw-2
    # with per-parity counts comfortably under the 8-tile boundary
    for c in range(cfg.n_cores):
        lo = c * own
        deg = indeg[lo:lo + own]
        order = np.argsort(-deg, kind="stable")
        spill = order[:128]
        members[c, nw - 1, :128] = lo + spill
        fill[c, nw - 1] = 128
        heap = [(0, w) for w in range(nw - 1)]
        heapq.heapify(heap)
        for i in order[128:]:
            while True:
                t, w = heapq.heappop(heap)
                if fill[c, w] < 128:
                    break
            members[c, w, fill[c, w]] = lo + i
            fill[c, w] += 1
            heapq.heappush(heap, (t + int(deg[i]), w))
    # tentative positions: members order; pairs are (slot 2i, 2i+1)
    # out-edge CSR by src node
    e_order = np.argsort(src, kind="stable")
    src_s = src[e_order]
    # global dst window id under the new window assignment needs dst pos;
    # dst windows are fixed by pass 1 (independent of pair parity)
    win_of_node = np.zeros(n, np.int64)
    for c in range(cfg.n_cores):
        for w in range(nw):
            m = members[c, w, :fill[c, w]]
            win_of_node[m] = c * nw + w
    out_w = win_of_node[dst[e_order]]
    starts = np.searchsorted(src_s, np.arange(n + 1))
    imb = np.zeros(cfg.n_cores * nw, np.int64)
    for c in range(cfg.n_cores):
        for w in range(nw):
            f = fill[c, w]
            m = members[c, w, :f]
            npair = (f // 2) * 2
            for i in range(0, npair, 2):
                a, b = m[i], m[i + 1]
                wa = out_w[starts[a]:starts[a + 1]]
                wb = out_w[starts[b]:starts[b + 1]]
                delta = imb[wa].sum() - imb[wb].sum()
                if delta > 0:   # put a at odd, b at even
                    members[c, w, i], members[c, w, i + 1] = b, a
                    wa, wb = wb, wa
                np.add.at(imb, wa, 1)
                np.add.at(imb, wb, -1)
            base = np.int64(w) * 128
            pos[members[c, w, :npair]] = base + np.arange(npair,
                                                          dtype=np.int64)
            if f % 2 == 1:
                # unpaired member: choose its slot parity via the pad gap
                a = m[f - 1]
                wa = out_w[starts[a]:starts[a + 1]]
                if imb[wa].sum() > 0 and f < 128:
                    pos[a] = base + f       # odd slot, gap at f-1
                    np.add.at(imb, wa, -1)
                else:
                    pos[a] = base + f - 1   # even slot
                    np.add.at(imb, wa, 1)
    return None, pos  # local positions


def host_prep(cfg, x, edge_attr, edge_index):
    src = edge_index[0].astype(np.int64)
    dst = edge_index[1].astype(np.int64)
    own, own_pad, nw = cfg.own, cfg.own_pad, cfg.n_win
    core_of = dst // own
    # chunk-major global table layout: [chunk][core][rows-in-chunk], so the
    # per-chunk AllGather output is contiguous
    CH = np.array(cfg.chunks, dtype=np.int64)
    if int(os.environ.get("BAL", "1")):
        _, pos_local = _balance_perm(cfg, src, dst)
    else:
        pos_local = np.arange(cfg.n_nodes, dtype=np.int64) % own
    cfg.pos_local = pos_local
    src_core = src // own
    r_local = pos_local[src]
    k = np.searchsorted(CH, r_local, side="right") - 1
    cw_k = CH[k + 1] - CH[k]
    tab_row = 8 * CH[k] + src_core * cw_k + (r_local - CH[k])

    win_of = pos_local[dst] // 128
    par_of = tab_row & 1
    # order: core -> window -> parity -> (stable)
    order = np.lexsort((par_of, win_of, core_of))
    dst_s = dst[order]
    tr_s = tab_row[order]
    ea_s = np.asarray(edge_attr)[order]
    x_s = np.asarray(x)[src[order]]
    core_s = core_of[order]
    win_s = win_of[order]
    par_s = par_of[order]

    # counts per (core, window, parity)
    cnt = np.zeros((cfg.n_cores, nw, 2), dtype=np.int64)
    np.add.at(cnt, (core_s, win_s, par_s), 1)
    # tiles per (core, window, parity), >= 1 so every window gets started
    tcnt = np.maximum(1, (cnt + 127) // 128)
    # uniform tile count across cores for a single SPMD program:
    # per (window, parity) take the max over cores
    tcnt_u = tcnt.max(axis=0)                      # [nw, 2]
    TE = int(tcnt_u[:, 0].sum())
    TO = int(tcnt_u[:, 1].sum())
    # pad each sweep's tile count to a multiple of 8 (block granularity)
    TE = ((TE + 7) // 8) * 8
    TO = ((TO + 7) // 8) * 8
    NT = TE + TO

    # schedule: even/odd sweeps interleaved at 4-window-group granularity
    # (E-g0, O-g0, E-g1, O-g1, ...) so aggT groups complete early and the
    # node chunks gate sooner; each (group, parity) segment is padded to a
    # multiple of 8 tiles so gather calls stay block- and parity-aligned
    n_grp0 = (nw + 3) // 4
    sched = []   # list of (window, parity)
    segments = []  # (tile0, ntiles, par)
    for g in range(n_grp0):
        wins = list(range(g * 4, min(g * 4 + 4, nw)))
        for par in (0, 1):
            s0 = len(sched)
            for w in wins:
                for _ in range(int(tcnt_u[w, par])):
                    sched.append((w, par))
            while (len(sched) - s0) % 8:
                sched.append((wins[-1], par))  # pad: all slots dead
            segments.append((s0, len(sched) - s0, par))
    NT = len(sched)
    TE = sum(s[1] for s in segments if s[2] == 0)
    TO = NT - TE

    # per-tile start flags (first tile of its window within the sweep) and
    # evac points (last tile of a bank group of 4 windows within the sweep)
    tiles_of = {}
    for t, (w, par) in enumerate(sched):
        tiles_of.setdefault((par, w), []).append(t)
    start_t = {min(v) for v in tiles_of.values()}
    n_grp = (nw + 3) // 4
    evac_t = {}   # tile -> (parity, group)
    for par in (0, 1):
        for g in range(n_grp):
            last = max(max(tiles_of[(par, w)]) for w in
                       range(g * 4, min(g * 4 + 4, nw)) if (par, w) in tiles_of)
            evac_t[last] = (par, g)

    idx16 = np.zeros((cfg.n_cores, 128, NT * 8), dtype=np.int16)
    dstrel_i16 = np.full((cfg.n_cores, 128, NT), -1, dtype=np.int16)
    # k-major stationary layouts: one 2KB descriptor per partition row on load
    ea_stat = np.zeros((cfg.n_cores, 6, NT, 128), dtype=BF16)
    xg_stat = np.zeros((cfg.n_cores, 11, NT, 128), dtype=BF16)
    # real (pre-8-align) tile counts per sweep for trailing-pad pruning
    real_TE = int(tcnt_u[:, 0].sum())
    real_TO = int(tcnt_u[:, 1].sum())

    starts = np.concatenate([[0], np.cumsum(cnt.reshape(-1))])
    for c in range(cfg.n_cores):
        gidx_flat = np.zeros((NT * 128,), np.int64)
        for par in (0, 1):
            for w in range(nw):
                tl = tiles_of[(par, w)]
                g = (c * nw + w) * 2 + par
                s0, s1 = starts[g], starts[g + 1]
                k = s1 - s0
                if k == 0:
                    continue
                assert k <= len(tl) * 128
                j = np.arange(k)
                t_ = np.array(tl)[j // 128]
                p_ = j % 128
                gidx_flat[t_ * 128 + p_] = tr_s[s0:s1] >> 1
                dstrel_i16[c, p_, t_] = pos_local[dst_s[s0:s1]] - 128 * w
                ea_stat[c, 0:5, t_, p_] = ea_s[s0:s1].astype(BF16)
                ea_stat[c, 5, t_, p_] = BF16(1.0)
                xg_stat[c, 0:5, t_, p_] = x_s[s0:s1].astype(BF16)
                xg_stat[c, 5:10, t_, p_] = ea_s[s0:s1].astype(BF16)
                xg_stat[c, 10, t_, p_] = BF16(1.0)
        idx16[c] = _wrap16(gidx_flat.astype(np.int16))

    xT = np.zeros((cfg.n_cores, 6, own_pad), dtype=np.float32)
    xnp = np.asarray(x, dtype=np.float32)
    for c in range(cfg.n_cores):
        vids = np.arange(c * own, (c + 1) * own)
        xT[c, 0:5, pos_local[vids]] = xnp[vids]
        xT[c, 5, pos_local[vids]] = 1.0

    per_core = [dict(idx16=idx16[c], dstrel=dstrel_i16[c], ea_stat=ea_stat[c],
                     xg_stat=xg_stat[c], xT=xT[c]) for c in range(cfg.n_cores)]
    meta = dict(TE=TE, TO=TO, NT=NT, real_TE=real_TE, real_TO=real_TO,
                segments=tuple(segments),
                sched=tuple(sched),
                start_t=frozenset(start_t),
                evac_t=tuple(sorted(evac_t.items())))
    return meta, per_core


def fold_params(p):
    """p: dict of raw params. Returns folded weight arrays."""
    inv_std = 1.0 / np.sqrt(1.0 + BN_EPS)
    Wp = [p["edge_W"] @ p["lin_W"][l] for l in range(N_LAYERS)]      # [5,64]
    bp = [p["edge_b"] @ p["lin_W"][l] + p["lin_b"][l] for l in range(N_LAYERS)]
    rhs_l1 = np.concatenate([p["node_W"], Wp[0],
                             (p["node_b"] + bp[0])[None, :]], axis=0)  # [11,64]
    rhs_c = [np.concatenate([Wp[l], bp[l][None, :]], axis=0)
             for l in range(1, N_LAYERS)]                              # [6,64]
    nwgt = np.concatenate([p["node_W"], p["node_b"][None, :]], axis=0)  # [6,64]
    w1 = [p["mlp_W1"][l] for l in range(N_LAYERS)]
    b1 = [p["mlp_b1"][l] for l in range(N_LAYERS)]
    s = [p["bn_g"][l] * inv_std for l in range(N_LAYERS)]
    w2 = [p["mlp_W2"][l] * s[l][None, :] for l in range(N_LAYERS)]
    b2 = [p["mlp_b2"][l] * s[l] + p["bn_b"][l] for l in range(N_LAYERS)]
    return dict(rhs_l1=rhs_l1.astype(BF16), rhs_c=[a.astype(BF16) for a in rhs_c],
                nwgt=nwgt.astype(np.float32),
                w1=[a.astype(np.float32) for a in w1],
                b1=[a.astype(np.float32).reshape(64, 1) for a in b1],
                w2=[a.astype(np.float32) for a in w2],
                b2=[a.astype(np.float32).reshape(64, 1) for a in b2])


# =============================================================== device build
def build_program(cfg, meta):
    import concourse.bacc as bacc
    import concourse.tile as tile
    from concourse import mybir

    f32 = mybir.dt.float32
    f32r = mybir.dt.float32r
    bf16 = mybir.dt.bfloat16
    i16 = mybir.dt.int16
    AT = mybir.ActivationFunctionType
    OP = mybir.AluOpType

    own_pad, nw = cfg.own_pad, cfg.n_win
    TE, TO, NT = meta["TE"], meta["TO"], meta["NT"]
    real_TE, real_TO = meta["real_TE"], meta["real_TO"]
    sched = meta["sched"]
    start_t = meta["start_t"]
    evac_t = dict(meta["evac_t"])
    NBLK = NT // 8
    tab_pairs = cfg.tab_rows // 2

    nc = bacc.Bacc(num_devices=cfg.n_cores)

    d_idx16 = nc.declare_dram_parameter("idx16", [128, NT * 8], i16, isOutput=False)
    d_dstrel = nc.declare_dram_parameter("dstrel", [128, NT], i16, isOutput=False)
    d_ea = nc.declare_dram_parameter("ea_stat", [6, NT, 128], bf16, isOutput=False)
    d_xg = nc.declare_dram_parameter("xg_stat", [11, NT, 128], bf16, isOutput=False)
    d_xT = nc.declare_dram_parameter("xT", [6, own_pad], f32, isOutput=False)
    d_rhs1 = nc.declare_dram_parameter("rhs_l1", [11, 64], bf16, isOutput=False)
    d_nw = nc.declare_dram_parameter("nwgt", [6, 64], f32, isOutput=False)
    d_rhsc = [nc.declare_dram_parameter(f"rhs_c{l}", [6, 64], bf16, isOutput=False)
              for l in range(1, N_LAYERS)]
    d_w1 = [nc.declare_dram_parameter(f"w1_{l}", [64, 64], f32, isOutput=False)
            for l in range(N_LAYERS)]
    d_b1 = [nc.declare_dram_parameter(f"b1_{l}", [64, 1], f32, isOutput=False)
            for l in range(N_LAYERS)]
    d_w2 = [nc.declare_dram_parameter(f"w2_{l}", [64, 64], f32, isOutput=False)
            for l in range(N_LAYERS)]
    d_b2 = [nc.declare_dram_parameter(f"b2_{l}", [64, 1], f32, isOutput=False)
            for l in range(N_LAYERS)]
    d_out = nc.declare_dram_parameter("hout", [64, own_pad], f32, isOutput=True)

    # flat bf16 tables; +128 elems so the odd pair-view's last row stays
    # in bounds
    d_htab = [nc.dram_tensor(f"htab{l}", [cfg.tab_rows * 64 + 128], bf16,
                             addr_space="Shared")
              for l in range(N_LAYERS - 1)]
    d_hown = [nc.dram_tensor(f"hown{l}", [own_pad, 64], bf16)
              for l in range(N_LAYERS - 1)]

    with tile.TileContext(nc) as tc:
        with tc.tile_pool(name="persist", bufs=1) as pp, \
             tc.tile_pool(name="stat", bufs=4) as statp, \
             tc.tile_pool(name="gath", bufs=4) as gathp, \
             tc.tile_pool(name="msg", bufs=4) as msgp, \
             tc.tile_pool(name="pre", bufs=4) as prep, \
             tc.tile_pool(name="oh", bufs=4) as ohp, \
             tc.tile_pool(name="trn", bufs=1) as trnp, \
             tc.tile_pool(name="cps", bufs=4, space="PSUM") as cpsump, \
             tc.tile_pool(name="aps", bufs=2, space="PSUM") as apsump, \
             tc.tile_pool(name="nps", bufs=2, space="PSUM") as npsump:

            # ------------------------------------------------ persistent loads
            # loads needed by the first edge blocks / h0 go first; bulky or
            # later-needed loads (idx16, rhsc, MLP weights) are emitted after
            # the h0 loop so the first stationary prefetches aren't queued
            # behind them on the sync DMA queue
            dstrel_t = pp.tile([128, NT], i16)
            nc.sync.dma_start(dstrel_t[:], d_dstrel[:])
            rhs1_t = pp.tile([11, 64], bf16)
            nc.sync.dma_start(rhs1_t[:], d_rhs1[:])
            nw_t = pp.tile([6, 64], f32)
            nc.sync.dma_start(nw_t[:], d_nw[:])
            xT_t = pp.tile([6, own_pad], f32)
            nc.sync.dma_start(xT_t[:], d_xT[:])
            iota_t = pp.tile([128, 8, 128], i16)
            nc.gpsimd.iota(iota_t[:], pattern=[[0, 8], [1, 128]], base=0,
                           channel_multiplier=0)
            # deferred persistent loads: emitted mid-L0 (after the first
            # stationary prefetches) so L0's first blocks aren't queued
            # behind these bulky loads on the sync DMA queue. First uses:
            # w/b at the first interleaved node chunk (~66% into L0),
            # idx16/rhsc at layer 1.
            idx16_t = None
            rhsc_t, w1_t, b1_t, w2_t, b2_t = [], [], [], [], []

            def emit_deferred_loads():
                nonlocal idx16_t
                idx16_t = pp.tile([128, NT * 8], i16)
                nc.sync.dma_start(idx16_t[:], d_idx16[:])
                for i, d in enumerate(d_rhsc):
                    t = pp.tile([6, 64], bf16, tag=f"rhsc{i}")
                    nc.sync.dma_start(t[:], d[:])
                    rhsc_t.append(t)
                for l in range(N_LAYERS):
                    t = pp.tile([64, 64], f32, tag=f"w1{l}")
                    nc.sync.dma_start(t[:], d_w1[l][:])
                    w1_t.append(t)
                    t = pp.tile([64, 1], f32, tag=f"bb1{l}")
                    nc.sync.dma_start(t[:], d_b1[l][:])
                    b1_t.append(t)
                    t = pp.tile([64, 64], f32, tag=f"w2{l}")
                    nc.sync.dma_start(t[:], d_w2[l][:])
                    w2_t.append(t)
                    t = pp.tile([64, 1], f32, tag=f"bb2{l}")
                    nc.sync.dma_start(t[:], d_b2[l][:])
                    b2_t.append(t)

            hT = pp.tile([64, own_pad], f32)      # current h^T
            aggT_e = pp.tile([64, own_pad], f32, tag="agg0")
            aggT_o = pp.tile([64, own_pad], f32, tag="agg1")
            aggT = [aggT_e, aggT_o]

            # ------------------------------------------------ h0^T
            for (a, b) in _chunks(own_pad, 512):
                ps = npsump.tile([64, 512], f32, tag="nps")
                nc.tensor.matmul(ps[:, 0:b - a], nw_t[:],
                                 xT_t[:, a:b],
                                 start=True, stop=True)
                nc.scalar.activation(hT[:, a:b], ps[:, 0:b - a], AT.Copy)

            # gather call ranges per segment: list of (tile0, ntiles, par)
            gcalls = []
            for (s0, slen, spar) in meta["segments"]:
                for (a, b) in _chunks(slen, GBT):
                    gcalls.append((s0 + a, b - a, spar))

            # ------------------------------------------------ layers
            ag_chunks = [(cfg.chunks[i], cfg.chunks[i + 1])
                         for i in range(len(cfg.chunks) - 1)]
            # node chunk ci may be emitted once all evacs covering its
            # columns (both sweeps) are emitted: gate = max evac tile index
            chunk_gate = []
            for (ca, cb) in ag_chunks:
                g0, g1 = ca // 512, (cb + 511) // 512
                gate = max(t for t, (p, g) in evac_t.items() if g0 <= g < g1)
                chunk_gate.append(gate)

            def emit_node_chunk(l, ci):
                """z = hT + aggE + aggO; 2-layer MLP (BN folded); for
                l < N_LAYERS-1 also cast/transpose/AllGather the h chunk
                into the next layer's gather table."""
                ca, cb = ag_chunks[ci]
                for (a, b) in [(ca + x, min(ca + x + 512, cb))
                               for x in range(0, cb - ca, 512)]:
                    nc.vector.tensor_tensor(aggT_e[:, a:b], aggT_e[:, a:b],
                                            aggT_o[:, a:b], OP.add)
                    nc.vector.tensor_tensor(aggT_e[:, a:b], aggT_e[:, a:b],
                                            hT[:, a:b], OP.add)
                    ps = npsump.tile([64, 512], f32, tag="nps")
                    nc.tensor.matmul(ps[:, 0:b - a], w1_t[l][:],
                                     aggT_e[:, a:b], start=True, stop=True)
                    nc.scalar.activation(aggT_o[:, a:b], ps[:, 0:b - a],
                                         AT.Relu, bias=b1_t[l][:])
                    ps = npsump.tile([64, 512], f32, tag="nps")
                    nc.tensor.matmul(ps[:, 0:b - a], w2_t[l][:],
                                     aggT_o[:, a:b], start=True, stop=True)
                    nc.scalar.activation(hT[:, a:b], ps[:, 0:b - a],
                                         AT.Relu, bias=b2_t[l][:])
                if l == N_LAYERS - 1:
                    nc.sync.dma_start(d_out[:, ca:cb], hT[:, ca:cb])
                if l < N_LAYERS - 1:
                    cw = cb - ca
                    hbf = trnp.tile([64, 2048], bf16, tag="hbf")
                    nc.vector.tensor_copy(hbf[:, 0:cw], hT[:, ca:cb])
                    hnm = trnp.tile([128, 2048 // 128, 64], bf16, tag="hnm")
                    nc.sync.dma_start_transpose(
                        hnm[:, 0:cw // 128, :], hbf[:, 0:cw])
                    nc.sync.dma_start(
                        d_hown[l][ca:cb, :]
                        .rearrange("(n p) f -> p n f", p=128),
                        hnm[:, 0:cw // 128, :])
                    nc.gpsimd.collective_compute(
                        "AllGather", OP.bypass,
                        replica_groups=[list(range(cfg.n_cores))],
                        ins=[d_hown[l][ca:cb, :]],
                        outs=[d_htab[l][8 * ca * 64:8 * cb * 64]
                              .rearrange("(a b) -> a b", b=64)],
                    )

            for l in range(N_LAYERS):
                # ---------------- edge phase (node chunks of THIS layer are
                # interleaved as soon as their aggT groups are evacuated, so
                # the MLP + AllGather chain hides under the gather span)
                cur_aps = None
                hg_buf = None
                hg_t0 = 0
                gci = 0
                next_chunk = 0
                for blk in range(NBLK):
                    t0 = blk * 8
                    # batched pair-view gather (layers >= 1)
                    if l > 0 and gci < len(gcalls) and gcalls[gci][0] == t0:
                        gt0, gnt, par = gcalls[gci]
                        gci += 1
                        hg_t0 = gt0
                        if par == 0:
                            view = d_htab[l - 1][0:tab_pairs * 128]\
                                .rearrange("(a b) -> a b", b=128)
                        else:
                            view = d_htab[l - 1][64:64 + tab_pairs * 128]\
                                .rearrange("(a b) -> a b", b=128)
                        hg_buf = gathp.tile([128, GBT, 128], bf16, tag="hg")
                        nc.gpsimd.dma_gather(
                            out_ap=hg_buf[:, 0:gnt, :], in_ap=view,
                            idxs_ap=idx16_t[:, gt0 * 8:(gt0 + gnt) * 8],
                            num_idxs=gnt * 128, num_idxs_reg=gnt * 128,
                            elem_size=128,
                            single_packet=bool(int(os.environ.get('SPK', '0'))))

                    if l == 0 and blk == 3:
                        emit_deferred_loads()

                    # stationary prefetch (k-major: contiguous 2KB per row)
                    K = 11 if l == 0 else 6
                    dsrc = d_xg if l == 0 else d_ea
                    st = statp.tile([K, 8, 128], bf16, tag="st")
                    nc.sync.dma_start(st[:], dsrc[:, t0:t0 + 8, :])

                    # pre-msg matmuls -> cpsum [128, 512]
                    cps = cpsump.tile([128, 512], f32, tag="cps")
                    wrhs = rhs1_t if l == 0 else rhsc_t[l - 1]
                    for i in range(8):
                        nc.tensor.matmul(cps[:, 64 * i:64 * i + 64],
                                         st[:, i, :], wrhs[:],
                                         start=True, stop=True)

                    m = msgp.tile([128, 8, 64], bf16, tag="m")
                    if l == 0:
                        nc.scalar.activation(
                            m[:].rearrange("p t f -> p (t f)"), cps[:], AT.Relu)
                    else:
                        # pre = hg (cols 0:64 of pair rows) + c, then relu
                        off = t0 - hg_t0
                        pre = prep.tile([128, 512], bf16, tag="pre")
                        nc.vector.tensor_tensor(
                            pre[:].rearrange("p (t f) -> p t f", f=64),
                            hg_buf[:, off:off + 8, 0:64],
                            cps[:].rearrange("p (t f) -> p t f", f=64),
                            OP.add)
                        nc.scalar.activation(
                            m[:].rearrange("p t f -> p (t f)"), pre[:], AT.Relu)

                    # one-hot [128, 8, 128] bf16
                    oh = ohp.tile([128, 8, 128], bf16, tag="oh")
                    nc.vector.tensor_tensor(
                        oh[:],
                        dstrel_t[:, t0:t0 + 8].rearrange("p (t o) -> p t o", o=1)
                        .to_broadcast([128, 8, 128]),
                        iota_t[:], OP.is_equal)

                    # scatter matmuls
                    for i in range(8):
                        t = t0 + i
                        w, par = sched[t]
                        grp = w // 4
                        col = 128 * (w % 4)
                        if t in start_t and w % 4 == 0:
                            cur_aps = apsump.tile([64, 512], f32, tag="aps")
                        aps = cur_aps
                        nc.tensor.matmul(
                            aps[:, col:col + 128], m[:, i, :], oh[:, i, :],
                            start=(t in start_t), stop=True)
                        if t in evac_t:
                            epar, g = evac_t[t]
                            a = 512 * g
                            b = min(a + 512, own_pad)
                            nc.scalar.activation(
                                aggT[epar][:, a:b], aps[:, 0:b - a], AT.Copy)

                    # interleave ready node chunks of this layer
                    while (next_chunk < len(ag_chunks)
                           and chunk_gate[next_chunk] <= t0 + 7):
                        emit_node_chunk(l, next_chunk)
                        next_chunk += 1

                # ---------------- remaining node chunks
                while next_chunk < len(ag_chunks):
                    emit_node_chunk(l, next_chunk)
                    next_chunk += 1


    nc.compile()
    return nc


# =============================================================== entry point
_CACHE = {}


def kernel(x, edge_attr, edge_index, batch, node_W, node_b, edge_W, edge_b,
           lin_W, lin_b, mlp_W1, mlp_b1, mlp_W2, mlp_b2, bn_g, bn_b,
           head_W1, head_b1, head_W2, head_b2):
    from concourse.bass_utils import run_bass_kernel_spmd

    x = np.asarray(x, dtype=np.float32)
    edge_attr = np.asarray(edge_attr, dtype=np.float32)
    edge_index = np.asarray(edge_index)
    batch_np = np.asarray(batch).astype(np.int64)

    cfg = Cfg(n_nodes=x.shape[0], n_cores=N_CORES)
    meta, per_core = host_prep(cfg, x, edge_attr, edge_index)
    params = {k: np.asarray(v, dtype=np.float32) for k, v in dict(
        node_W=node_W, node_b=node_b, edge_W=edge_W, edge_b=edge_b,
        lin_W=lin_W, lin_b=lin_b, mlp_W1=mlp_W1, mlp_b1=mlp_b1,
        mlp_W2=mlp_W2, mlp_b2=mlp_b2, bn_g=bn_g, bn_b=bn_b).items()}
    fold = fold_params(params)

    key = (cfg.n_nodes, meta["TE"], meta["TO"], meta["sched"],
           meta["start_t"], meta["evac_t"])
    if key not in _CACHE:
        _CACHE[key] = build_program(cfg, meta)
    nc = _CACHE[key]

    common = dict(rhs_l1=fold["rhs_l1"], nwgt=fold["nwgt"])
    for i, a in enumerate(fold["rhs_c"]):
        common[f"rhs_c{i + 1}"] = a
    for l in range(N_LAYERS):
        common[f"w1_{l}"] = fold["w1"][l]
        common[f"b1_{l}"] = fold["b1"][l]
        common[f"w2_{l}"] = fold["w2"][l]
        common[f"b2_{l}"] = fold["b2"][l]

    in_maps = []
    for c in range(cfg.n_cores):
        m = dict(common)
        m.update(per_core[c])
        in_maps.append(m)

    trace = bool(int(os.environ.get("GNN_TRACE", "0")))
    if trace:
        trace = _install_ntff_shim()
    res = run_bass_kernel_spmd(nc, in_maps, core_ids=list(range(cfg.n_cores)),
                               trace=trace)
    kernel._last_results = res

    # assemble h3 [n_nodes, 64]
    h3 = np.zeros((cfg.n_nodes, HID), dtype=np.float32)
    for c in range(cfg.n_cores):
        hout = np.asarray(res.results[c]["hout"], dtype=np.float32)  # [64, own_pad]
        vids = np.arange(c * cfg.own, (c + 1) * cfg.own)
        h3[vids] = hout[:, cfg.pos_local[vids]].T

    # pooling + head on host (exact fp32, tiny)
    G = int(batch_np.max()) + 1 if batch_np.size else 0
    G = max(G, N_GRAPHS)
    counts = np.zeros((G,), np.float32)
    np.add.at(counts, batch_np, 1.0)
    h_sum = np.zeros((G, HID), np.float32)
    np.add.at(h_sum, batch_np, h3)
    h_mean = h_sum / np.maximum(counts, 1.0)[:, None]
    h_max = np.full((G, HID), -np.inf, np.float32)
    np.maximum.at(h_max, batch_np, h3)
    h_max = np.where(counts[:, None] > 0, h_max, 0.0)
    hc = np.concatenate([h_mean, h_max, h_sum], axis=-1)
    hw1 = np.asarray(head_W1, np.float32)
    hb1 = np.asarray(head_b1, np.float32)
    hw2 = np.asarray(head_W2, np.float32)
    hb2 = np.asarray(head_b2, np.float32)
    out = np.maximum(hc @ hw1 + hb1, 0.0) @ hw2 + hb2
    return out.astype(np.float32)



# revision 47
# speedup vs baseline: 1.0280x; 1.0280x over previous
"""Trainium2 Bass kernel for nn_CGNN (3-layer GINE-style message-passing GNN).

Self-contained: takes FULL inputs (as produced by the problem's setup_inputs),
distributes across 8 NeuronCores internally (SPMD, one program, per-core data),
returns the FULL [64, 5] output.

Per-core design (v3 — batched SWDGE gathers + balanced node permutation):
  - nodes split into 8 equal ranges of N/8 (padded to a multiple of 128);
    each edge lives on the core owning its dst node.
  - per-core node permutation (_balance_perm): LPT-packs nodes into 128-node
    dst windows balancing indegree, then picks pair parities by greedy
    discrepancy so each (window, src-parity) class needs ~8 edge tiles
    (NT 880 -> 848); applied to table rows, dst windows, xT and h3.
  - edges grouped by 128-node dst windows; within each window edges are split
    by src parity (even/odd) and padded to 128-edge tiles. A layer's edge
    phase runs as two sweeps: all even tiles (window-major), then all odd
    tiles. Each sweep accumulates into its own SBUF aggT buffer, so PSUM
    bank-group lifetimes stay local to the sweep.
  - h table in DRAM is bf16 [n_nodes_pad, 64], viewed as pair rows
    [n/2, 128] (256B stride). Gathers use nc.gpsimd.dma_gather with int16
    indices = src>>1 (fits int16): even-src tiles gather from the base view,
    odd-src tiles from the +64-element offset view. GBT=8 tiles per call
    (994ns fixed SWDGE overhead amortized; 1024 idx = SWDGE ring max;
    measured ~8.6us/call = the Q7 descriptor-generation floor, gathp bufs=4
    keeps calls streaming).
  - stationary edge features staged k-major [K, NT, 128] so each block load
    is one 2KB descriptor per partition row (keeps SDMA off the gather path).
  - per edge tile: c = [ea; 1] @ [W'l; b'l] via per-tile PE matmul into a
    [128, 512] PSUM block (8 tiles); gathered h added via DVE; ACT relu ->
    m bf16.
  - scatter: aggT[:, win] += m^T via PE matmul with DVE-built one-hot.
  - node phase, feat-major: z-adds chunked into the MLP loop; MLP with BN
    folded into W2/b2; bias+relu fused in ACT. (f32r matmuls break the
    axon-path neuronxcc compile — keep fp32.)
  - next-layer table: bf16 cast + DMA-transpose + AllGather per chunk
    (0,2048,4096,5760,6272) — small last chunk so the final AllGather,
    which gates the next layer's gathers, lands early. The per-layer
    AllGather chain is serialized on the CC path (~81us).
  - pooling + the tiny 2-layer head run on host from returned h3 slices.
"""
import os
import sys
import numpy as np

sys.path.insert(0, "/opt/trn_rl_repo")

import ml_dtypes  # noqa: E402


def _install_ntff_shim(so_path="/opt/axon/libaxon_pjrt.so"):
    """Register the axon NTFF profile hook so trace=True works (optional)."""
    import types, contextlib, ctypes
    try:
        lib = ctypes.CDLL(so_path)
        if not hasattr(lib, "axon_start_nrt_profile"):
            return False
        lib.axon_start_nrt_profile.argtypes = [ctypes.POINTER(ctypes.c_int64),
                                               ctypes.c_size_t]
        lib.axon_start_nrt_profile.restype = ctypes.c_int64
        lib.axon_stop_nrt_profile.argtypes = [ctypes.c_char_p]
        lib.axon_stop_nrt_profile.restype = ctypes.c_int64

        @contextlib.contextmanager
        def _hook(output_dir, device_ids):
            import jax
            jax.devices()
            if device_ids:
                ids = (ctypes.c_int64 * len(device_ids))(*device_ids)
                rc = lib.axon_start_nrt_profile(ids, len(device_ids))
            else:
                rc = lib.axon_start_nrt_profile(None, 0)
            if rc != 0:
                raise RuntimeError(f"axon_start_nrt_profile rc={rc}")
            try:
                yield
            finally:
                n = lib.axon_stop_nrt_profile(str(output_dir).encode())
                if n < 0:
                    raise RuntimeError(f"axon_stop_nrt_profile rc={n}")

        import antenv
        mod = types.ModuleType("antenv.axon_hooks")
        mod.get_axon_ntff_profile_hook = lambda: _hook
        mod.set_axon_ntff_profile_hook = lambda h: None
        sys.modules["antenv.axon_hooks"] = mod
        antenv.axon_hooks = mod
        return True
    except Exception:
        return False

N_NODES = 50000
HID = 64
N_LAYERS = 3
N_CLASSES = 5
N_GRAPHS = 64
BN_EPS = 1e-5
N_CORES = 8
GBT = int(__import__('os').environ.get('GBT', 8))  # tiles per batched dma_gather call

BF16 = ml_dtypes.bfloat16


class Cfg:
    def __init__(self, n_nodes=N_NODES, n_cores=N_CORES):
        assert n_nodes % n_cores == 0
        self.n_nodes = n_nodes
        self.n_cores = n_cores
        self.own = n_nodes // n_cores
        self.own_pad = ((self.own + 127) // 128) * 128
        self.n_win = self.own_pad // 128
        self.tab_rows = n_cores * self.own_pad  # even; +2 pad rows in tensor
        # chunk-major table layout boundaries == AllGather chunks; last chunk
        # small so the final AllGather (which gates the next layer's gathers)
        # completes right after the node phase
        assert self.own_pad == 6272
        self.chunks = (0, 2048, 4096, 5760, self.own_pad)


def _chunks(total, step):
    out, a = [], 0
    while a < total:
        out.append((a, min(a + step, total)))
        a += step
    return out


def _wrap16(idx_flat):
    """SWDGE index layout: index g at [g%16, g//16], replicated over the
    8 gpsimd core groups (partitions 16a+r)."""
    n = idx_flat.shape[0]
    a = np.zeros((16, n // 16), np.int16)
    a[np.arange(n) % 16, np.arange(n) // 16] = idx_flat
    return np.tile(a, (8, 1))


# =============================================================== host prep
def _balance_perm(cfg, src, dst):
    """Per-core node permutation: LPT-pack nodes into 128-node windows so
    per-window indegree is balanced, then choose pair parities (greedy
    discrepancy) so each window's even/odd source split is balanced. Nearly
    every (window, parity) class then needs exactly 8 edge tiles.
    Returns pos: node -> local position (0..own_pad-1) within its core."""
    import heapq
    own, own_pad, nw = cfg.own, cfg.own_pad, cfg.n_win
    n = cfg.n_nodes
    indeg = np.bincount(dst, minlength=n)
    pos = np.zeros(n, np.int64)
    members = np.full((cfg.n_cores, nw, 128), -1, np.int64)
    fill = np.zeros((cfg.n_cores, nw), np.int64)
    # last window = spill: the 128 highest-indegree nodes (aligned across
    # cores so the per-class max stays tight); LPT the rest into 0..nw-2
    # with per-parity counts comfortably under the 8-tile boundary
    for c in range(cfg.n_cores):
        lo = c * own
        deg = indeg[lo:lo + own]
        order = np.argsort(-deg, kind="stable")
        spill = order[:128]
        members[c, nw - 1, :128] = lo + spill
        fill[c, nw - 1] = 128
        heap = [(0, w) for w in range(nw - 1)]
        heapq.heapify(heap)
        for i in order[128:]:
            while True:
                t, w = heapq.heappop(heap)
                if fill[c, w] < 128:
                    break
            members[c, w, fill[c, w]] = lo + i
            fill[c, w] += 1
            heapq.heappush(heap, (t + int(deg[i]), w))
    # tentative positions: members order; pairs are (slot 2i, 2i+1)
    # out-edge CSR by src node
    e_order = np.argsort(src, kind="stable")
    src_s = src[e_order]
    # global dst window id under the new window assignment needs dst pos;
    # dst windows are fixed by pass 1 (independent of pair parity)
    win_of_node = np.zeros(n, np.int64)
    for c in range(cfg.n_cores):
        for w in range(nw):
            m = members[c, w, :fill[c, w]]
            win_of_node[m] = c * nw + w
    out_w = win_of_node[dst[e_order]]
    starts = np.searchsorted(src_s, np.arange(n + 1))
    imb = np.zeros(cfg.n_cores * nw, np.int64)
    for c in range(cfg.n_cores):
        for w in range(nw):
            f = fill[c, w]
            m = members[c, w, :f]
            npair = (f // 2) * 2
            for i in range(0, npair, 2):
                a, b = m[i], m[i + 1]
                wa = out_w[starts[a]:starts[a + 1]]
                wb = out_w[starts[b]:starts[b + 1]]
                delta = imb[wa].sum() - imb[wb].sum()
                if delta > 0:   # put a at odd, b at even
                    members[c, w, i], members[c, w, i + 1] = b, a
                    wa, wb = wb, wa
                np.add.at(imb, wa, 1)
                np.add.at(imb, wb, -1)
            base = np.int64(w) * 128
            pos[members[c, w, :npair]] = base + np.arange(npair,
                                                          dtype=np.int64)
            if f % 2 == 1:
                # unpaired member: choose its slot parity via the pad gap
                a = m[f - 1]
                wa = out_w[starts[a]:starts[a + 1]]
                if imb[wa].sum() > 0 and f < 128:
                    pos[a] = base + f       # odd slot, gap at f-1
                    np.add.at(imb, wa, -1)
                else:
                    pos[a] = base + f - 1   # even slot
                    np.add.at(imb, wa, 1)
    return None, pos  # local positions


def host_prep(cfg, x, edge_attr, edge_index):
    src = edge_index[0].astype(np.int64)
    dst = edge_index[1].astype(np.int64)
    own, own_pad, nw = cfg.own, cfg.own_pad, cfg.n_win
    core_of = dst // own
    # chunk-major global table layout: [chunk][core][rows-in-chunk], so the
    # per-chunk AllGather output is contiguous
    CH = np.array(cfg.chunks, dtype=np.int64)
    if int(os.environ.get("BAL", "1")):
        _, pos_local = _balance_perm(cfg, src, dst)
    else:
        pos_local = np.arange(cfg.n_nodes, dtype=np.int64) % own
    cfg.pos_local = pos_local
    src_core = src // own
    r_local = pos_local[src]
    k = np.searchsorted(CH, r_local, side="right") - 1
    cw_k = CH[k + 1] - CH[k]
    tab_row = 8 * CH[k] + src_core * cw_k + (r_local - CH[k])

    win_of = pos_local[dst] // 128
    par_of = tab_row & 1
    # order: core -> window -> parity -> (stable)
    order = np.lexsort((par_of, win_of, core_of))
    dst_s = dst[order]
    tr_s = tab_row[order]
    ea_s = np.asarray(edge_attr)[order]
    x_s = np.asarray(x)[src[order]]
    core_s = core_of[order]
    win_s = win_of[order]
    par_s = par_of[order]

    # counts per (core, window, parity)
    cnt = np.zeros((cfg.n_cores, nw, 2), dtype=np.int64)
    np.add.at(cnt, (core_s, win_s, par_s), 1)
    # tiles per (core, window, parity), >= 1 so every window gets started
    tcnt = np.maximum(1, (cnt + 127) // 128)
    # uniform tile count across cores for a single SPMD program:
    # per (window, parity) take the max over cores
    tcnt_u = tcnt.max(axis=0)                      # [nw, 2]
    TE = int(tcnt_u[:, 0].sum())
    TO = int(tcnt_u[:, 1].sum())
    # pad each sweep's tile count to a multiple of 8 (block granularity)
    TE = ((TE + 7) // 8) * 8
    TO = ((TO + 7) // 8) * 8
    NT = TE + TO

    # schedule: per sweep, window-major tile list
    # tile metadata (same for all cores): window id, start flag, evac flag
    sched = []   # list of (window, parity)
    for par, base, tot in ((0, 0, TE), (1, TE, TO)):
        for w in range(nw):
            for _ in range(int(tcnt_u[w, par])):
                sched.append((w, par))
        while len(sched) - base < tot:
            sched.append((nw - 1, par))  # pad tiles: window irrelevant (all
                                         # slots dead), keep last window
    assert len(sched) == NT

    # per-tile start flags (first tile of its window within the sweep) and
    # evac points (last tile of a bank group of 4 windows within the sweep)
    tiles_of = {}
    for t, (w, par) in enumerate(sched):
        tiles_of.setdefault((par, w), []).append(t)
    start_t = {min(v) for v in tiles_of.values()}
    n_grp = (nw + 3) // 4
    evac_t = {}   # tile -> (parity, group)
    for par in (0, 1):
        for g in range(n_grp):
            last = max(max(tiles_of[(par, w)]) for w in
                       range(g * 4, min(g * 4 + 4, nw)) if (par, w) in tiles_of)
            evac_t[last] = (par, g)

    idx16 = np.zeros((cfg.n_cores, 128, NT * 8), dtype=np.int16)
    dstrel_i16 = np.full((cfg.n_cores, 128, NT), -1, dtype=np.int16)
    # k-major stationary layouts: one 2KB descriptor per partition row on load
    ea_stat = np.zeros((cfg.n_cores, 6, NT, 128), dtype=BF16)
    xg_stat = np.zeros((cfg.n_cores, 11, NT, 128), dtype=BF16)
    # real (pre-8-align) tile counts per sweep for trailing-pad pruning
    real_TE = int(tcnt_u[:, 0].sum())
    real_TO = int(tcnt_u[:, 1].sum())

    starts = np.concatenate([[0], np.cumsum(cnt.reshape(-1))])
    for c in range(cfg.n_cores):
        gidx_flat = np.zeros((NT * 128,), np.int64)
        for par in (0, 1):
            for w in range(nw):
                tl = tiles_of[(par, w)]
                g = (c * nw + w) * 2 + par
                s0, s1 = starts[g], starts[g + 1]
                k = s1 - s0
                if k == 0:
                    continue
                assert k <= len(tl) * 128
                j = np.arange(k)
                t_ = np.array(tl)[j // 128]
                p_ = j % 128
                gidx_flat[t_ * 128 + p_] = tr_s[s0:s1] >> 1
                dstrel_i16[c, p_, t_] = pos_local[dst_s[s0:s1]] - 128 * w
                ea_stat[c, 0:5, t_, p_] = ea_s[s0:s1].astype(BF16)
                ea_stat[c, 5, t_, p_] = BF16(1.0)
                xg_stat[c, 0:5, t_, p_] = x_s[s0:s1].astype(BF16)
                xg_stat[c, 5:10, t_, p_] = ea_s[s0:s1].astype(BF16)
                xg_stat[c, 10, t_, p_] = BF16(1.0)
        idx16[c] = _wrap16(gidx_flat.astype(np.int16))

    xT = np.zeros((cfg.n_cores, 6, own_pad), dtype=np.float32)
    xnp = np.asarray(x, dtype=np.float32)
    for c in range(cfg.n_cores):
        vids = np.arange(c * own, (c + 1) * own)
        xT[c, 0:5, pos_local[vids]] = xnp[vids]
        xT[c, 5, pos_local[vids]] = 1.0

    per_core = [dict(idx16=idx16[c], dstrel=dstrel_i16[c], ea_stat=ea_stat[c],
                     xg_stat=xg_stat[c], xT=xT[c]) for c in range(cfg.n_cores)]
    meta = dict(TE=TE, TO=TO, NT=NT, real_TE=real_TE, real_TO=real_TO,
                sched=tuple(sched),
                start_t=frozenset(start_t),
                evac_t=tuple(sorted(evac_t.items())))
    return meta, per_core


def fold_params(p):
    """p: dict of raw params. Returns folded weight arrays."""
    inv_std = 1.0 / np.sqrt(1.0 + BN_EPS)
    Wp = [p["edge_W"] @ p["lin_W"][l] for l in range(N_LAYERS)]      # [5,64]
    bp = [p["edge_b"] @ p["lin_W"][l] + p["lin_b"][l] for l in range(N_LAYERS)]
    rhs_l1 = np.concatenate([p["node_W"], Wp[0],
                             (p["node_b"] + bp[0])[None, :]], axis=0)  # [11,64]
    rhs_c = [np.concatenate([Wp[l], bp[l][None, :]], axis=0)
             for l in range(1, N_LAYERS)]                              # [6,64]
    nwgt = np.concatenate([p["node_W"], p["node_b"][None, :]], axis=0)  # [6,64]
    w1 = [p["mlp_W1"][l] for l in range(N_LAYERS)]
    b1 = [p["mlp_b1"][l] for l in range(N_LAYERS)]
    s = [p["bn_g"][l] * inv_std for l in range(N_LAYERS)]
    w2 = [p["mlp_W2"][l] * s[l][None, :] for l in range(N_LAYERS)]
    b2 = [p["mlp_b2"][l] * s[l] + p["bn_b"][l] for l in range(N_LAYERS)]
    return dict(rhs_l1=rhs_l1.astype(BF16), rhs_c=[a.astype(BF16) for a in rhs_c],
                nwgt=nwgt.astype(np.float32),
                w1=[a.astype(np.float32) for a in w1],
                b1=[a.astype(np.float32).reshape(64, 1) for a in b1],
                w2=[a.astype(np.float32) for a in w2],
                b2=[a.astype(np.float32).reshape(64, 1) for a in b2])


# =============================================================== device build
def build_program(cfg, meta):
    import concourse.bacc as bacc
    import concourse.tile as tile
    from concourse import mybir

    f32 = mybir.dt.float32
    f32r = mybir.dt.float32r
    bf16 = mybir.dt.bfloat16
    i16 = mybir.dt.int16
    AT = mybir.ActivationFunctionType
    OP = mybir.AluOpType

    own_pad, nw = cfg.own_pad, cfg.n_win
    TE, TO, NT = meta["TE"], meta["TO"], meta["NT"]
    real_TE, real_TO = meta["real_TE"], meta["real_TO"]
    sched = meta["sched"]
    start_t = meta["start_t"]
    evac_t = dict(meta["evac_t"])
    NBLK = NT // 8
    tab_pairs = cfg.tab_rows // 2

    nc = bacc.Bacc(num_devices=cfg.n_cores)

    d_idx16 = nc.declare_dram_parameter("idx16", [128, NT * 8], i16, isOutput=False)
    d_dstrel = nc.declare_dram_parameter("dstrel", [128, NT], i16, isOutput=False)
    d_ea = nc.declare_dram_parameter("ea_stat", [6, NT, 128], bf16, isOutput=False)
    d_xg = nc.declare_dram_parameter("xg_stat", [11, NT, 128], bf16, isOutput=False)
    d_xT = nc.declare_dram_parameter("xT", [6, own_pad], f32, isOutput=False)
    d_rhs1 = nc.declare_dram_parameter("rhs_l1", [11, 64], bf16, isOutput=False)
    d_nw = nc.declare_dram_parameter("nwgt", [6, 64], f32, isOutput=False)
    d_rhsc = [nc.declare_dram_parameter(f"rhs_c{l}", [6, 64], bf16, isOutput=False)
              for l in range(1, N_LAYERS)]
    d_w1 = [nc.declare_dram_parameter(f"w1_{l}", [64, 64], f32, isOutput=False)
            for l in range(N_LAYERS)]
    d_b1 = [nc.declare_dram_parameter(f"b1_{l}", [64, 1], f32, isOutput=False)
            for l in range(N_LAYERS)]
    d_w2 = [nc.declare_dram_parameter(f"w2_{l}", [64, 64], f32, isOutput=False)
            for l in range(N_LAYERS)]
    d_b2 = [nc.declare_dram_parameter(f"b2_{l}", [64, 1], f32, isOutput=False)
            for l in range(N_LAYERS)]
    d_out = nc.declare_dram_parameter("hout", [64, own_pad], f32, isOutput=True)

    # flat bf16 tables; +128 elems so the odd pair-view's last row stays
    # in bounds
    d_htab = [nc.dram_tensor(f"htab{l}", [cfg.tab_rows * 64 + 128], bf16,
                             addr_space="Shared")
              for l in range(N_LAYERS - 1)]
    d_hown = [nc.dram_tensor(f"hown{l}", [own_pad, 64], bf16)
              for l in range(N_LAYERS - 1)]

    with tile.TileContext(nc) as tc:
        with tc.tile_pool(name="persist", bufs=1) as pp, \
             tc.tile_pool(name="stat", bufs=6) as statp, \
             tc.tile_pool(name="gath", bufs=4) as gathp, \
             tc.tile_pool(name="msg", bufs=6) as msgp, \
             tc.tile_pool(name="pre", bufs=6) as prep, \
             tc.tile_pool(name="oh", bufs=6) as ohp, \
             tc.tile_pool(name="trn", bufs=1) as trnp, \
             tc.tile_pool(name="cps", bufs=4, space="PSUM") as cpsump, \
             tc.tile_pool(name="aps", bufs=2, space="PSUM") as apsump, \
             tc.tile_pool(name="nps", bufs=2, space="PSUM") as npsump:

            # ------------------------------------------------ persistent loads
            # loads needed by the first edge blocks / h0 go first; bulky or
            # later-needed loads (idx16, rhsc, MLP weights) are emitted after
            # the h0 loop so the first stationary prefetches aren't queued
            # behind them on the sync DMA queue
            dstrel_t = pp.tile([128, NT], i16)
            nc.sync.dma_start(dstrel_t[:], d_dstrel[:])
            rhs1_t = pp.tile([11, 64], bf16)
            nc.sync.dma_start(rhs1_t[:], d_rhs1[:])
            nw_t = pp.tile([6, 64], f32)
            nc.sync.dma_start(nw_t[:], d_nw[:])
            xT_t = pp.tile([6, own_pad], f32)
            nc.sync.dma_start(xT_t[:], d_xT[:])
            iota_t = pp.tile([128, 8, 128], i16)
            nc.gpsimd.iota(iota_t[:], pattern=[[0, 8], [1, 128]], base=0,
                           channel_multiplier=0)
            # deferred persistent loads: emitted mid-L0 (after the first
            # stationary prefetches) so L0's first blocks aren't queued
            # behind these bulky loads on the sync DMA queue. First uses:
            # w/b at the first interleaved node chunk (~66% into L0),
            # idx16/rhsc at layer 1.
            idx16_t = None
            rhsc_t, w1_t, b1_t, w2_t, b2_t = [], [], [], [], []

            def emit_deferred_loads():
                nonlocal idx16_t
                idx16_t = pp.tile([128, NT * 8], i16)
                nc.sync.dma_start(idx16_t[:], d_idx16[:])
                for i, d in enumerate(d_rhsc):
                    t = pp.tile([6, 64], bf16, tag=f"rhsc{i}")
                    nc.sync.dma_start(t[:], d[:])
                    rhsc_t.append(t)
                for l in range(N_LAYERS):
                    t = pp.tile([64, 64], f32, tag=f"w1{l}")
                    nc.sync.dma_start(t[:], d_w1[l][:])
                    w1_t.append(t)
                    t = pp.tile([64, 1], f32, tag=f"bb1{l}")
                    nc.sync.dma_start(t[:], d_b1[l][:])
                    b1_t.append(t)
                    t = pp.tile([64, 64], f32, tag=f"w2{l}")
                    nc.sync.dma_start(t[:], d_w2[l][:])
                    w2_t.append(t)
                    t = pp.tile([64, 1], f32, tag=f"bb2{l}")
                    nc.sync.dma_start(t[:], d_b2[l][:])
                    b2_t.append(t)

            hT = pp.tile([64, own_pad], f32)      # current h^T
            aggT_e = pp.tile([64, own_pad], f32, tag="agg0")
            aggT_o = pp.tile([64, own_pad], f32, tag="agg1")
            aggT = [aggT_e, aggT_o]

            # ------------------------------------------------ h0^T
            for (a, b) in _chunks(own_pad, 512):
                ps = npsump.tile([64, 512], f32, tag="nps")
                nc.tensor.matmul(ps[:, 0:b - a], nw_t[:],
                                 xT_t[:, a:b],
                                 start=True, stop=True)
                nc.scalar.activation(hT[:, a:b], ps[:, 0:b - a], AT.Copy)

            # gather call ranges per sweep: list of (tile0, ntiles)
            gcalls = []
            for base, tot in ((0, TE), (TE, TO)):
                for (a, b) in _chunks(tot, GBT):
                    gcalls.append((base + a, b - a))

            # ------------------------------------------------ layers
            ag_chunks = [(cfg.chunks[i], cfg.chunks[i + 1])
                         for i in range(len(cfg.chunks) - 1)]
            # node chunk ci may be emitted once all evacs covering its
            # columns (both sweeps) are emitted: gate = max evac tile index
            chunk_gate = []
            for (ca, cb) in ag_chunks:
                g0, g1 = ca // 512, (cb + 511) // 512
                gate = max(t for t, (p, g) in evac_t.items() if g0 <= g < g1)
                chunk_gate.append(gate)

            def emit_node_chunk(l, ci):
                """z = hT + aggE + aggO; 2-layer MLP (BN folded); for
                l < N_LAYERS-1 also cast/transpose/AllGather the h chunk
                into the next layer's gather table."""
                ca, cb = ag_chunks[ci]
                for (a, b) in [(ca + x, min(ca + x + 512, cb))
                               for x in range(0, cb - ca, 512)]:
                    nc.vector.tensor_tensor(aggT_e[:, a:b], aggT_e[:, a:b],
                                            aggT_o[:, a:b], OP.add)
                    nc.vector.tensor_tensor(aggT_e[:, a:b], aggT_e[:, a:b],
                                            hT[:, a:b], OP.add)
                    ps = npsump.tile([64, 512], f32, tag="nps")
                    nc.tensor.matmul(ps[:, 0:b - a], w1_t[l][:],
                                     aggT_e[:, a:b], start=True, stop=True)
                    nc.scalar.activation(aggT_o[:, a:b], ps[:, 0:b - a],
                                         AT.Relu, bias=b1_t[l][:])
                    ps = npsump.tile([64, 512], f32, tag="nps")
                    nc.tensor.matmul(ps[:, 0:b - a], w2_t[l][:],
                                     aggT_o[:, a:b], start=True, stop=True)
                    nc.scalar.activation(hT[:, a:b], ps[:, 0:b - a],
                                         AT.Relu, bias=b2_t[l][:])
                if l == N_LAYERS - 1:
                    nc.sync.dma_start(d_out[:, ca:cb], hT[:, ca:cb])
                if l < N_LAYERS - 1:
                    cw = cb - ca
                    hbf = trnp.tile([64, 2048], bf16, tag="hbf")
                    nc.vector.tensor_copy(hbf[:, 0:cw], hT[:, ca:cb])
                    hnm = trnp.tile([128, 2048 // 128, 64], bf16, tag="hnm")
                    nc.sync.dma_start_transpose(
                        hnm[:, 0:cw // 128, :], hbf[:, 0:cw])
                    nc.sync.dma_start(
                        d_hown[l][ca:cb, :]
                        .rearrange("(n p) f -> p n f", p=128),
                        hnm[:, 0:cw // 128, :])
                    nc.gpsimd.collective_compute(
                        "AllGather", OP.bypass,
                        replica_groups=[list(range(cfg.n_cores))],
                        ins=[d_hown[l][ca:cb, :]],
                        outs=[d_htab[l][8 * ca * 64:8 * cb * 64]
                              .rearrange("(a b) -> a b", b=64)],
                    )

            for l in range(N_LAYERS):
                # ---------------- edge phase (node chunks of THIS layer are
                # interleaved as soon as their aggT groups are evacuated, so
                # the MLP + AllGather chain hides under the gather span)
                cur_aps = None
                hg_buf = None
                hg_t0 = 0
                gci = 0
                next_chunk = 0
                for blk in range(NBLK):
                    t0 = blk * 8
                    # batched pair-view gather (layers >= 1)
                    if l > 0 and gci < len(gcalls) and gcalls[gci][0] == t0:
                        gt0, gnt = gcalls[gci]
                        gci += 1
                        hg_t0 = gt0
                        par = 0 if gt0 < TE else 1
                        if par == 0:
                            view = d_htab[l - 1][0:tab_pairs * 128]\
                                .rearrange("(a b) -> a b", b=128)
                        else:
                            view = d_htab[l - 1][64:64 + tab_pairs * 128]\
                                .rearrange("(a b) -> a b", b=128)
                        hg_buf = gathp.tile([128, GBT, 128], bf16, tag="hg")
                        nc.gpsimd.dma_gather(
                            out_ap=hg_buf[:, 0:gnt, :], in_ap=view,
                            idxs_ap=idx16_t[:, gt0 * 8:(gt0 + gnt) * 8],
                            num_idxs=gnt * 128, num_idxs_reg=gnt * 128,
                            elem_size=128,
                            single_packet=bool(int(os.environ.get('SPK', '0'))))

                    if l == 0 and blk == 3:
                        emit_deferred_loads()

                    # stationary prefetch (k-major: contiguous 2KB per row)
                    K = 11 if l == 0 else 6
                    dsrc = d_xg if l == 0 else d_ea
                    st = statp.tile([K, 8, 128], bf16, tag="st")
                    nc.sync.dma_start(st[:], dsrc[:, t0:t0 + 8, :])

                    # pre-msg matmuls -> cpsum [128, 512]
                    cps = cpsump.tile([128, 512], f32, tag="cps")
                    wrhs = rhs1_t if l == 0 else rhsc_t[l - 1]
                    for i in range(8):
                        nc.tensor.matmul(cps[:, 64 * i:64 * i + 64],
                                         st[:, i, :], wrhs[:],
                                         start=True, stop=True)

                    m = msgp.tile([128, 8, 64], bf16, tag="m")
                    if l == 0:
                        nc.scalar.activation(
                            m[:].rearrange("p t f -> p (t f)"), cps[:], AT.Relu)
                    else:
                        # pre = hg (cols 0:64 of pair rows) + c, then relu
                        off = t0 - hg_t0
                        pre = prep.tile([128, 512], bf16, tag="pre")
                        nc.vector.tensor_tensor(
                            pre[:].rearrange("p (t f) -> p t f", f=64),
                            hg_buf[:, off:off + 8, 0:64],
                            cps[:].rearrange("p (t f) -> p t f", f=64),
                            OP.add)
                        nc.scalar.activation(
                            m[:].rearrange("p t f -> p (t f)"), pre[:], AT.Relu)

                    # one-hot [128, 8, 128] bf16
                    oh = ohp.tile([128, 8, 128], bf16, tag="oh")
                    nc.vector.tensor_tensor(
                        oh[:],
                        dstrel_t[:, t0:t0 + 8].rearrange("p (t o) -> p t o", o=1)
                        .to_broadcast([128, 8, 128]),
                        iota_t[:], OP.is_equal)

                    # scatter matmuls
                    for i in range(8):
                        t = t0 + i
                        w, par = sched[t]
                        grp = w // 4
                        col = 128 * (w % 4)
                        if t in start_t and w % 4 == 0:
                            cur_aps = apsump.tile([64, 512], f32, tag="aps")
                        aps = cur_aps
                        nc.tensor.matmul(
                            aps[:, col:col + 128], m[:, i, :], oh[:, i, :],
                            start=(t in start_t), stop=True)
                        if t in evac_t:
                            epar, g = evac_t[t]
                            a = 512 * g
                            b = min(a + 512, own_pad)
                            nc.scalar.activation(
                                aggT[epar][:, a:b], aps[:, 0:b - a], AT.Copy)

                    # interleave ready node chunks of this layer
                    while (next_chunk < len(ag_chunks)
                           and chunk_gate[next_chunk] <= t0 + 7):
                        emit_node_chunk(l, next_chunk)
                        next_chunk += 1

                # ---------------- remaining node chunks
                while next_chunk < len(ag_chunks):
                    emit_node_chunk(l, next_chunk)
                    next_chunk += 1


    nc.compile()
    return nc


# =============================================================== entry point
_CACHE = {}


def kernel(x, edge_attr, edge_index, batch, node_W, node_b, edge_W, edge_b,
           lin_W, lin_b, mlp_W1, mlp_b1, mlp_W2, mlp_b2, bn_g, bn_b,
           head_W1, head_b1, head_W2, head_b2):
    from concourse.bass_utils import run_bass_kernel_spmd

    x = np.asarray(x, dtype=np.float32)
    edge_attr = np.asarray(edge_attr, dtype=np.float32)
    edge_index = np.asarray(edge_index)
    batch_np = np.asarray(batch).astype(np.int64)

    cfg = Cfg(n_nodes=x.shape[0], n_cores=N_CORES)
    meta, per_core = host_prep(cfg, x, edge_attr, edge_index)
    params = {k: np.asarray(v, dtype=np.float32) for k, v in dict(
        node_W=node_W, node_b=node_b, edge_W=edge_W, edge_b=edge_b,
        lin_W=lin_W, lin_b=lin_b, mlp_W1=mlp_W1, mlp_b1=mlp_b1,
        mlp_W2=mlp_W2, mlp_b2=mlp_b2, bn_g=bn_g, bn_b=bn_b).items()}
    fold = fold_params(params)

    key = (cfg.n_nodes, meta["TE"], meta["TO"], meta["sched"],
           meta["start_t"], meta["evac_t"])
    if key not in _CACHE:
        _CACHE[key] = build_program(cfg, meta)
    nc = _CACHE[key]

    common = dict(rhs_l1=fold["rhs_l1"], nwgt=fold["nwgt"])
    for i, a in enumerate(fold["rhs_c"]):
        common[f"rhs_c{i + 1}"] = a
    for l in range(N_LAYERS):
        common[f"w1_{l}"] = fold["w1"][l]
        common[f"b1_{l}"] = fold["b1"][l]
        common[f"w2_{l}"] = fold["w2"][l]
        common[f"b2_{l}"] = fold["b2"][l]

    in_maps = []
    for c in range(cfg.n_cores):
        m = dict(common)
        m.update(per_core[c])
        in_maps.append(m)

    trace = bool(int(os.environ.get("GNN_TRACE", "0")))
    if trace:
        trace = _install_ntff_shim()
    res = run_bass_kernel_spmd(nc, in_maps, core_ids=list(range(cfg.n_cores)),
                               trace=trace)
    kernel._last_results = res

    # assemble h3 [n_nodes, 64]
    h3 = np.zeros((cfg.n_nodes, HID), dtype=np.float32)
    for c in range(cfg.n_cores):
        hout = np.asarray(res.results[c]["hout"], dtype=np.float32)  # [64, own_pad]
        vids = np.arange(c * cfg.own, (c + 1) * cfg.own)
        h3[vids] = hout[:, cfg.pos_local[vids]].T

    # pooling + head on host (exact fp32, tiny)
    G = int(batch_np.max()) + 1 if batch_np.size else 0
    G = max(G, N_GRAPHS)
    counts = np.zeros((G,), np.float32)
    np.add.at(counts, batch_np, 1.0)
    h_sum = np.zeros((G, HID), np.float32)
    np.add.at(h_sum, batch_np, h3)
    h_mean = h_sum / np.maximum(counts, 1.0)[:, None]
    h_max = np.full((G, HID), -np.inf, np.float32)
    np.maximum.at(h_max, batch_np, h3)
    h_max = np.where(counts[:, None] > 0, h_max, 0.0)
    hc = np.concatenate([h_mean, h_max, h_sum], axis=-1)
    hw1 = np.asarray(head_W1, np.float32)
    hb1 = np.asarray(head_b1, np.float32)
    hw2 = np.asarray(head_W2, np.float32)
    hb2 = np.asarray(head_b2, np.float32)
    out = np.maximum(hc @ hw1 + hb1, 0.0) @ hw2 + hb2
    return out.astype(np.float32)

